# revision 50
# baseline (speedup 1.0000x reference)
"""Trainium2 Bass kernel for DeiT self-attention with channel-pruning masks.

Reference computation (B=16, S=577, HID=768, H=12, D=64, N_KEEP=576):
    q/k/v = hs @ W + b            [B,S,576]
    scatter channels to [B,S,768] at {q,k,v}_idx, split into 12 heads of 64
    softmax attention per (b, h), concat heads, gather v_idx channels.

Strategy:
  - Host folds the channel scatters into the weight matrices (zero columns
    at dropped channels), so the device kernel is dense attention over the
    full 768-channel layout. hs is pre-transposed per core on the host.
  - Data-parallel over batch: 8 cores x 2 images each (T = 1154 tokens/core).
  - All matmul operands are bf16 (PSUM accumulation stays fp32; bf16 enables
    fast weight loads and halves DMA/SBUF). Outputs are written bf16; the
    host upcasts, divides by the softmax denominators, transposes, gathers.
  - Score matmuls contract over d=64, so heads 2i (SBUF partitions 0-63) and
    2i+1 (partitions 64-127) are emitted back-to-back: the PE array runs
    them concurrently as two 64x128 row tiles (tile_position auto-derived
    from operand base partitions) - measured ~2x on the score phase.
  - Score PSUM is a single 4-bank tile per (head-pair, key-chunk) group:
    [T0 qt0 | T0 qt1 | T8 qt0 | T8 qt1] at 512-col offsets, drained by ONE
    merged exp (ScalarE) per group (60 exps instead of 120; each exp's
    write-after-read round trip paces the score loop at ~1.9us/group).
  - Schedule: score groups are spread one-per-gap between projection / ctx
    chains so the PE stays busy during each exp round trip: P0 interleaves
    the first chunk-0 groups between chunk-0 projections, iteration 0
    carries all V chains, iteration i emits scores(i), Q/K projections
    (i+1), ctx pair-B(i-1) in the gaps and ctx pair-A(i) at the end.
"""

import numpy as np

B, S, HID = 16, 577, 768
H, D = 12, 64
N_KEEP = 576
NCORES = 8
BPC = B // NCORES          # images per core
TOK = BPC * S              # tokens per core
VW = H * (D + 1)           # 780: V columns augmented with per-head ones column
P = 128
ICH = HID // P             # 6 input-channel chunks
OCH = HID // P             # 6 q/k output-channel chunks
TOK_TILES = [(0, 386), (386, 386), (772, 382)]      # projection moving tiles
KCHUNKS = [(0, 128), (128, 128), (256, 128), (384, 128), (512, 65)]  # per image
NK = len(KCHUNKS)
# (q_offset, ctx mm width, output cols) per query tile; scores run as one
# 580-wide moving stream per (head, kchunk), qt1 covers queries 290-576
QTILES = [(0, 290, 290), (290, 288, 287)]
TOK_P = 1160                                        # q/k token dim padded for qt1 reads

_NC_CACHE = {}


def _build_nc():
    import concourse.bacc as bacc
    import concourse.mybir as mybir
    import concourse.tile as tile

    f32 = mybir.dt.float32
    bf16 = mybir.dt.bfloat16

    nc = bacc.Bacc("TRN2", target_bir_lowering=False)

    hsT = nc.dram_tensor("hsT", [HID, TOK], bf16, kind="ExternalInput")
    # host-swizzled: wq[p, i, c, n] = Wq_full[c*128+p, i*128+n] so each
    # chunk-i slice is one contiguous-descriptor DMA
    wq = nc.dram_tensor("wq", [P, OCH, ICH, P], bf16, kind="ExternalInput")
    wk = nc.dram_tensor("wk", [P, OCH, ICH, P], bf16, kind="ExternalInput")
    wv = nc.dram_tensor("wv", [HID, VW], bf16, kind="ExternalInput")
    bq = nc.dram_tensor("bq", [HID], f32, kind="ExternalInput")
    bk = nc.dram_tensor("bk", [HID], f32, kind="ExternalInput")
    bvb = nc.dram_tensor("bvb", [P, VW], f32, kind="ExternalInput")
    outA = nc.dram_tensor("outA", [VW, TOK], bf16, kind="ExternalOutput")

    def mm(out_ps, lhsT, rhs, start, stop):
        nc.tensor.matmul(out_ps, lhsT, rhs, start=start, stop=stop)

    with tile.TileContext(nc) as tc:
        Exp = mybir.ActivationFunctionType.Exp
        with (
            tc.tile_pool(name="big", bufs=1) as big,
            tc.tile_pool(name="pscore", bufs=1, space="PSUM") as pscore,  # 4 banks
            tc.tile_pool(name="psa", bufs=3, space="PSUM") as psa,       # proj/V/ctx accums
            tc.tile_pool(name="wpool", bufs=3) as wpool,
            tc.tile_pool(name="epool", bufs=6) as epool,
            tc.tile_pool(name="opool", bufs=4) as opool,
        ):
            # ---- persistent SBUF tensors ----
            hsT_sb = big.tile([P, ICH, TOK], bf16)
            hsT_r = hsT.rearrange("(c p) t -> p c t", p=P)
            bvb_sb = big.tile([P, VW], f32)
            bq_sb = big.tile([P, OCH], f32)
            bk_sb = big.tile([P, OCH], f32)

            q_sb = big.tile([P, OCH, TOK_P], bf16)
            k_sb = big.tile([P, OCH, TOK], bf16)
            v_sb = big.tile([P, BPC * NK, VW], bf16)
            # zero the padded token tail once (read by qt1 score matmuls for b=1)
            nc.vector.memset(q_sb[:, :, TOK:].bitcast(f32), 0.0)

            # score PSUM: one 4-bank tile per (head-pair, kchunk) group:
            # [T0 qt0 @0, T0 qt1 @512, T8 qt0 @1024, T8 qt1 @1536]
            score_ps = pscore.tile([P, 2048], f32)
            score_v = score_ps.rearrange("p (four q) -> p four q", four=4)

            # ---- Q0/K0 projections first: small weight slices + hsT only,
            # so the PE starts while the bulk inputs stream ----
            def load_w(i, w_dram):
                w_sb = wpool.tile([P, ICH, P], bf16, tag="w", name="w_sb")
                nc.sync.dma_start(w_sb[:], w_dram[:, i, :, :])
                return w_sb

            wq0 = load_w(0, wq)
            wk0 = load_w(0, wk)
            nc.sync.dma_start(bq_sb[:], bq.rearrange("(c p) -> p c", p=P))
            nc.sync.dma_start(bk_sb[:], bk.rearrange("(c p) -> p c", p=P))
            # per-(chunk, token-tile) pieces on parallel queues; the first six
            # unblock the t0 projection tile as early as possible
            for toff, tcs in TOK_TILES:
                for c in range(ICH):
                    nc.sync.dma_start(
                        hsT_sb[:, c, toff : toff + tcs],
                        hsT_r[:, c, toff : toff + tcs],
                    )

            def emit_proj_t(i, w_sb, b_sb, dst, t):
                toff, tcs = TOK_TILES[t]
                qp = psa.tile([P, 512], f32, tag="ps", name="qp")[:, :tcs]
                for k in range(ICH):
                    mm(
                        qp,
                        w_sb[:, k, :],
                        hsT_sb[:, k, toff : toff + tcs],
                        start=(k == 0),
                        stop=(k == ICH - 1),
                    )
                nc.vector.tensor_add(
                    out=dst[:, i, toff : toff + tcs],
                    in0=qp,
                    in1=b_sb[:, i : i + 1].to_broadcast((P, tcs)),
                )

            p0_order = [
                (wq0, bq_sb, q_sb, 0),
                (wk0, bk_sb, k_sb, 0),
                (wq0, bq_sb, q_sb, 1),
                (wk0, bk_sb, k_sb, 1),
                (wq0, bq_sb, q_sb, 2),
                (wk0, bk_sb, k_sb, 2),
            ]

            # ---- score group: heads (2i, 2i+1) of image b, one key chunk.
            # 4 matmuls: T0/T8 row tiles emitted adjacently so they overlap
            # in the PE array; one merged exp drains all 4 banks.
            def emit_score_group(i, b, c, e_pair):
                ko, kcs = KCHUNKS[c]
                for qt in range(2):
                    for u in range(2):  # u=0 -> head 2i (T0), u=1 -> 2i+1 (T8)
                        pb = 64 * u
                        mm(
                            score_v[:kcs, 2 * u + qt, :290],
                            k_sb[pb : pb + 64, i, b * S + ko : b * S + ko + kcs],
                            q_sb[pb : pb + 64, i, b * S + 290 * qt : b * S + 290 * qt + 290],
                            start=True,
                            stop=True,
                        )
                nc.scalar.activation(
                    e_pair[:kcs, c].rearrange("p u t q -> p (u t) q"),
                    score_v[:kcs, :, :290],
                    Exp,
                    scale=0.125,
                )

            # e_pair layout: [keys, kchunk, unit(2), qt(2), 290]
            def new_epair():
                return epool.tile([P, NK, 2, 2, 290], bf16, tag="e", name="e_pair")

            def emit_ctx(i, u, b, e_pair, qt):
                h = 2 * i + u
                qo, cw, ow = QTILES[qt]
                cp = psa.tile([P, 512], f32, tag="ps", name="cp")[:65, :cw]
                for c, (ko, kcs) in enumerate(KCHUNKS):
                    mm(
                        cp,
                        v_sb[:kcs, b * NK + c, h * 65 : (h + 1) * 65],
                        e_pair[:kcs, c, u, qt, :cw],
                        start=(c == 0),
                        stop=(c == NK - 1),
                    )
                o_sb = opool.tile([65, 512], bf16, tag="o", name="o_sb")[:, :cw]
                nc.vector.tensor_copy(o_sb, cp)
                nc.sync.dma_start(
                    outA[h * 65 : (h + 1) * 65, b * S + qo : b * S + qo + ow],
                    o_sb[:, :ow],
                )

            # ---- V projection pieces (emitted as iteration-0 fillers) ----
            wv_sb = big.tile([P, ICH, VW], bf16)
            wv_r = wv.rearrange("(c p) n -> p c n", p=P)
            for k in range(ICH):
                nc.sync.dma_start(wv_sb[:, k, :], wv_r[:, k, :])
            nc.sync.dma_start(bvb_sb[:], bvb[:])
            VT = VW // 2  # 390, head-aligned (6 heads x 65)

            def emit_v_chain(b, j):
                koff, kcs = KCHUNKS[j]
                toff = b * S + koff
                vps = [
                    psa.tile([P, 512], f32, tag="ps", name="vp")[:kcs, :VT]
                    for _ in range(2)
                ]
                for k in range(ICH):
                    for n in range(2):
                        mm(
                            vps[n],
                            hsT_sb[:, k, toff : toff + kcs],
                            wv_sb[:, k, n * VT : (n + 1) * VT],
                            start=(k == 0),
                            stop=(k == ICH - 1),
                        )
                for n in range(2):
                    nc.vector.tensor_add(
                        out=v_sb[:kcs, b * NK + j, n * VT : (n + 1) * VT],
                        in0=vps[n],
                        in1=bvb_sb[:kcs, n * VT : (n + 1) * VT],
                    )

            # ---- P0: chunk-0 projections with the first pair-A score groups
            # interleaved as soon as their q/k token tiles are ready, so the
            # exp pipeline starts during the cold-start region ----
            e0A = new_epair()
            e0B = new_epair()
            for t in range(4):
                emit_proj_t(0, *p0_order[t])
            emit_score_group(0, 0, 0, e0A)
            emit_proj_t(0, *p0_order[4])
            emit_score_group(0, 0, 1, e0A)
            emit_proj_t(0, *p0_order[5])
            emit_score_group(0, 0, 2, e0A)

            # ---- steady chunk loop ----
            # iteration i emits: score groups for chunk i (10: pair A = img 0,
            # pair B = img 1), Q/K projections for chunk i+1, ctx for
            # pair-B(i-1) and pair-A(i), plus all V chains during iteration 0.
            e_tiles = {0: (e0A, e0B)}

            def get_e(ch, b):
                if ch not in e_tiles:
                    e_tiles[ch] = (new_epair(), new_epair())
                return e_tiles[ch][b]

            for i in range(OCH):
                last = i + 1 >= OCH
                wqn = None if last else load_w(i + 1, wq)
                wkn = None if last else load_w(i + 1, wk)
                # iteration 0 additionally absorbs chunk-1's first three score
                # groups (its V fillers leave the exp pipeline under-used);
                # chunk i>=1 then starts its remaining groups at g=3.
                if i == 0:
                    glist = [(0, g) for g in range(3, 2 * NK)] + [
                        (1, g) for g in range(NK)]
                elif not last:
                    glist = [(i, g) for g in range(NK, 2 * NK)] + [
                        (i + 1, g) for g in range(NK)]
                else:
                    glist = [(i, g) for g in range(NK, 2 * NK)]
                eA, eB = get_e(i, 0), get_e(i, 1)

                def sg(idx):
                    ch, g = glist[idx]
                    bb, cc = divmod(g, NK)
                    emit_score_group(ch, bb, cc, get_e(ch, bb))

                pjq = [
                    lambda t=t: emit_proj_t(i + 1, wqn, bq_sb, q_sb, t)
                    for t in range(3)
                ]
                pjk = [
                    lambda t=t: emit_proj_t(i + 1, wkn, bk_sb, k_sb, t)
                    for t in range(3)
                ]
                cpB = [
                    lambda n=n: emit_ctx(i - 1, n // 2, 1, e_tiles[i - 1][1], n % 2)
                    for n in range(4)
                ]
                cpA = [
                    lambda n=n: emit_ctx(i, n // 2, 0, eA, n % 2)
                    for n in range(4)
                ]
                if i == 0:
                    # V chains interleaved with chunk-1 projections; image 0
                    # first (needed by ctx pair-A(0)); spread over all gaps,
                    # cpA(0) at the end.
                    vch = [
                        lambda b=b, j=j: emit_v_chain(b, j)
                        for b in range(BPC)
                        for j in range(NK)
                    ]
                    pjs = [pjq[0], pjq[1], pjk[0], pjk[1], pjq[2], pjk[2]]
                    fillers = []
                    for idx in range(16):
                        if idx in (2, 4, 6, 8, 10, 12) and pjs:
                            fillers.append(pjs.pop(0))
                        else:
                            fillers.append(vch.pop(0))
                    fillers += pjs + vch
                    nf = len(fillers)
                    ng = len(glist)
                    fi = 0
                    for gi in range(ng):
                        sg(gi)
                        take = (nf * (gi + 1)) // ng - fi
                        for _ in range(take):
                            fillers[fi]()
                            fi += 1
                    for f in cpA:
                        f()
                else:
                    # even round-robin of proj/ctx-B fillers across all gaps;
                    # pair-A ctx chains after the last group
                    if not last:
                        # pjk t0/t1 early: the donated pair-A(i+1) groups in
                        # the last five gaps read k(i+1) token tiles t0/t1
                        fillers = [pjq[0], pjq[1], cpB[0], pjk[0], cpB[1],
                                   pjk[1], cpB[2], pjq[2], cpB[3], pjk[2]]
                    else:
                        fillers = cpB
                    nf = len(fillers)
                    ng = len(glist)
                    fi = 0
                    for gi in range(ng):
                        sg(gi)
                        take = (nf * (gi + 1)) // ng - fi
                        for _ in range(take):
                            fillers[fi]()
                            fi += 1
                    for f in cpA:
                        f()

            # tail: ctx pair-B of the last chunk
            for n in range(4):
                emit_ctx(OCH - 1, n // 2, 1, e_tiles[OCH - 1][1], n % 2)

    nc.compile()
    return nc


def _get_nc():
    if "nc" not in _NC_CACHE:
        _NC_CACHE["nc"] = _build_nc()
    return _NC_CACHE["nc"]


def _to_bf16(x):
    import ml_dtypes

    return np.asarray(x, np.float32).astype(ml_dtypes.bfloat16)


def _make_in_maps(hidden_states, Wq, bq, Wk, bk, Wv, bv, q_idx, k_idx, v_idx):
    f32 = np.float32
    hs = np.asarray(hidden_states, f32)
    q_idx = np.asarray(q_idx).astype(np.int64)
    k_idx = np.asarray(k_idx).astype(np.int64)
    v_idx = np.asarray(v_idx).astype(np.int64)

    # fold channel scatters into full-width weights
    wq_full = np.zeros((HID, HID), f32)
    wq_full[:, q_idx] = np.asarray(Wq, f32)
    bq_full = np.zeros(HID, f32)
    bq_full[q_idx] = np.asarray(bq, f32)
    wk_full = np.zeros((HID, HID), f32)
    wk_full[:, k_idx] = np.asarray(Wk, f32)
    bk_full = np.zeros(HID, f32)
    bk_full[k_idx] = np.asarray(bk, f32)

    wv_full = np.zeros((HID, HID), f32)
    wv_full[:, v_idx] = np.asarray(Wv, f32)
    bv_full = np.zeros(HID, f32)
    bv_full[v_idx] = np.asarray(bv, f32)
    # augmented V layout: per head 64 value cols + a ones column (softmax denom)
    wv_aug = np.zeros((HID, VW), f32)
    bv_aug = np.zeros(VW, f32)
    for h in range(H):
        wv_aug[:, h * 65 : h * 65 + 64] = wv_full[:, h * 64 : (h + 1) * 64]
        bv_aug[h * 65 : h * 65 + 64] = bv_full[h * 64 : (h + 1) * 64]
        bv_aug[h * 65 + 64] = 1.0
    bvb = np.broadcast_to(bv_aug, (P, VW)).copy()

    # swizzle projection weights to [p, i, c, n] (slice-contiguous DMA layout)
    wq_sw = np.ascontiguousarray(
        wq_full.reshape(ICH, P, OCH, P).transpose(1, 2, 0, 3)
    )
    wk_sw = np.ascontiguousarray(
        wk_full.reshape(ICH, P, OCH, P).transpose(1, 2, 0, 3)
    )
    wq_b = _to_bf16(wq_sw)
    wk_b = _to_bf16(wk_sw)
    wv_b = _to_bf16(wv_aug)

    in_maps = []
    for c in range(NCORES):
        hsT = np.ascontiguousarray(
            hs[c * BPC : (c + 1) * BPC].reshape(TOK, HID).T
        )
        in_maps.append(
            {
                "hsT": _to_bf16(hsT),
                "wq": wq_b,
                "wk": wk_b,
                "wv": wv_b,
                "bq": bq_full,
                "bk": bk_full,
                "bvb": bvb,
            }
        )
    return in_maps, v_idx


def _assemble_output(results, v_idx):
    ctx = np.empty((B, S, HID), np.float32)
    for c in range(NCORES):
        aug = np.asarray(results[c]["outA"], dtype=np.float32).reshape(H, D + 1, TOK)
        ctxu = aug[:, :D, :] / aug[:, D:, :]            # [H, D, TOK]
        ctx[c * BPC : (c + 1) * BPC] = (
            ctxu.reshape(HID, TOK).T.reshape(BPC, S, HID)
        )
    return np.ascontiguousarray(ctx[:, :, v_idx])


def run(inputs, trace=False, **spmd_kwargs):
    """Full pipeline; returns (output, BassKernelResults)."""
    from concourse import bass_utils

    in_maps, v_idx = _make_in_maps(**inputs)
    nc = _get_nc()
    res = bass_utils.run_bass_kernel_spmd(
        nc, in_maps, core_ids=list(range(NCORES)), trace=trace, **spmd_kwargs
    )
    return _assemble_output(res.results, v_idx), res


def kernel(**inputs):
    out, _ = run(inputs, trace=False)
    return out
